# revision 35
# baseline (speedup 1.0000x reference)
"""Trainium2 Bass kernel for Cross-MultiAttention.

Problem (hardcoded shapes):
  B=4, T=2048, S=2048, C=256, E=512, H=8 heads, D=64, SCALE=E**-0.5
  xe  = x @ w_in.T + b_in                  [B,T,C] -> [B,T,E]
  Q   = xe @ wq.T + bq;  K/V from context  [B,S,E]
  att = softmax(mask(QK^T * SCALE))        [B,H,T,S]
  out = (att @ V) @ w_out.T + b_out        -> [B,T,C]

Sharding: 8 cores. Core c handles batch c//2 and head group c%2 (4 heads),
for the full T=2048 query range (head/tensor parallel within each batch
pair). proj_in (xe) is replicated within a pair; Q/K/V/proj_out use the
core's head-slice of the weights; proj_out emits a PARTIAL [C, T] result
(row-sharded w_out) and the host sums the pair's partials.

Device algorithm per core (bf16 matmul inputs / fp32 PSUM accumulate):
  xeT  [E,T]    = w_inT.T @ xT   (+b_in via ACT bias)
  QT   [256,T]  = wqT.T @ xeT    (+bq)   (local heads' e-range)
  KT   [256,S]  = wkT.T @ ctxT   (+bk)
  V    [S,4*65] = ctxT.T @ wvT   (+bv via K=1 ones matmul; col h*65+64 is an
                  all-ones denominator column, memset once)
  per (head h<4, T-chunk tc<2, key-tile j<16):
     logitsT = KT_h_j.T @ QT_h_tc          (PE, [128, 1024])
     P  = exp(SCALE * logitsT)             (ACT; no max pass needed:
                                            |SCALE*logits| < ~1, masking is
                                            multiplicative)
     Pm = P * zT[j]                        (DVE; z = 1-mask bf16)
     oav[0:65] += V_j_h.T @ Pm             (PE; row 64 = softmax denominator)
  normalize: rec = 1/oav[64] (DVE); rb = bcast(rec) (GPSIMD); ocat = oav*rb
  finT [C,T] (partial) = w_outT.T @ ocat (+b_out on head-group-0 cores only)

Q/K projections for the second half of local heads run as background
granules, one PE matmul per attention j-step, to fill PE slack during the
ACT(exp)-bound attention window.
"""

from collections import deque

import numpy as np
import ml_dtypes

import concourse.bass as bass
import concourse.tile as tile
import concourse.mybir as mybir
from concourse.bacc import Bacc
from concourse.bass_utils import run_bass_kernel_spmd

BF16 = mybir.dt.bfloat16
F32 = mybir.dt.float32
AF = mybir.ActivationFunctionType

B, T, S, C, E, H = 4, 2048, 2048, 256, 512, 8
D = E // H
SCALE = float(E) ** -0.5
NCORES = 8
HL = H // 2                    # 4 local heads per core
EL = HL * D                    # 256 local head-dims
NJ = S // 128                  # 16 key tiles
KE = E // 128                  # 4 contraction tiles over E
KC = C // 128                  # 2 contraction tiles over C
ML = EL // 128                 # 2 local m-tiles (Q/K out)
MC = C // 128                  # 2 output c-tiles
NTC = T // 1024                # 2 attention T-chunks
NT5 = T // 512                 # 4 512-chunks
HW = HL * 65                   # 260 V columns incl. denominator cols

_NC_CACHE = {}


def _build_nc(repeat=1):
    nc = Bacc("TRN2", target_bir_lowering=False, debug=False)

    xT = nc.dram_tensor("xT", [128, KC, T], BF16, kind="ExternalInput")
    ctxT = nc.dram_tensor("ctxT", [128, KE, S], BF16, kind="ExternalInput")
    zT = nc.dram_tensor("zT", [128, NJ, T], BF16, kind="ExternalInput")
    w_inT = nc.dram_tensor("w_inT", [128, KC, E], BF16, kind="ExternalInput")
    wqT = nc.dram_tensor("wqT", [128, KE, EL], BF16, kind="ExternalInput")
    wkT = nc.dram_tensor("wkT", [128, KE, EL], BF16, kind="ExternalInput")
    wvT = nc.dram_tensor("wvT", [128, KE, EL], BF16, kind="ExternalInput")
    wvb = nc.dram_tensor("wvb", [1, EL], BF16, kind="ExternalInput")
    w_outT = nc.dram_tensor("w_outT", [64, HL, C], BF16, kind="ExternalInput")
    wob = nc.dram_tensor("wob", [1, C], BF16, kind="ExternalInput")
    b_in = nc.dram_tensor("b_in", [128, KE], F32, kind="ExternalInput")
    bq = nc.dram_tensor("bq", [128, ML], F32, kind="ExternalInput")
    bk = nc.dram_tensor("bk", [128, ML], F32, kind="ExternalInput")
    outT = nc.dram_tensor("outT", [128, MC, T], F32, kind="ExternalOutput")

    with tile.TileContext(nc) as tc:
        with tc.tile_pool(name="const", bufs=1) as cp, \
             tc.tile_pool(name="acts", bufs=1) as ap, \
             tc.tile_pool(name="pp", bufs=5) as pp, \
             tc.tile_pool(name="nrm", bufs=2) as nrm, \
             tc.tile_pool(name="mm", bufs=3, space="PSUM") as ps_mm, \
             tc.tile_pool(name="av", bufs=2, space="PSUM") as ps_av:

            # ---- persistent loads (early-needed first; zT big and last) ----
            w_inT_sb = cp.tile([128, KC, E], BF16, tag="w_inT")
            nc.sync.dma_start(out=w_inT_sb, in_=w_inT[:, :, :])
            xT_sb = cp.tile([128, KC, T], BF16, tag="xT")
            nc.sync.dma_start(out=xT_sb[:, :, 0:1024], in_=xT[:, :, 0:1024])
            nc.sync.dma_start(out=xT_sb[:, :, 1024:T], in_=xT[:, :, 1024:T])
            b_in_sb = cp.tile([128, KE], F32, tag="b_in")
            nc.sync.dma_start(out=b_in_sb, in_=b_in[:, :])
            bq_sb = cp.tile([128, ML], F32, tag="bq")
            nc.sync.dma_start(out=bq_sb, in_=bq[:, :])
            bk_sb = cp.tile([128, ML], F32, tag="bk")
            nc.sync.dma_start(out=bk_sb, in_=bk[:, :])
            wqT_sb = cp.tile([128, KE, EL], BF16, tag="wqT")
            nc.sync.dma_start(out=wqT_sb, in_=wqT[:, :, :])
            wkT_sb = cp.tile([128, KE, EL], BF16, tag="wkT")
            nc.sync.dma_start(out=wkT_sb, in_=wkT[:, :, :])
            ctxT_sb = cp.tile([128, KE, S], BF16, tag="ctxT")
            nc.sync.dma_start(out=ctxT_sb, in_=ctxT[:, :, :])
            wvT_sb = cp.tile([128, KE, EL], BF16, tag="wvT")
            nc.sync.dma_start(out=wvT_sb, in_=wvT[:, :, :])
            wvb_sb = cp.tile([1, EL], BF16, tag="wvb")
            nc.sync.dma_start(out=wvb_sb, in_=wvb[:, :])
            w_outT_sb = cp.tile([64, HL, C], BF16, tag="w_outT")
            nc.sync.dma_start(out=w_outT_sb, in_=w_outT[:, :, :])
            wob_sb = cp.tile([1, C], BF16, tag="wob")
            nc.sync.dma_start(out=wob_sb, in_=wob[:, :])
            # zT streamed per key-tile: TT(j) only needs chunk j, so the
            # attention window can start long before the full 8MB lands
            zT_sb = cp.tile([128, NJ, T], BF16, tag="zT")
            for j in range(NJ):
                nc.sync.dma_start(out=zT_sb[:, j, :], in_=zT[:, j, :])

            ones128 = cp.tile([1, 128], BF16, tag="ones128")
            nc.vector.memset(ones128, 1.0)
            onest = cp.tile([1, 512], BF16, tag="onest")
            nc.vector.memset(onest, 1.0)

            # ---- persistent activations ----
            xeT_sb = ap.tile([128, KE, T], BF16, tag="xeT")
            QT_sb = ap.tile([128, ML, T], BF16, tag="QT")
            KT_sb = ap.tile([128, ML, S], BF16, tag="KT")
            V_sb = ap.tile([128, NJ, HW], BF16, tag="V")
            nc.vector.memset(
                V_sb[:, :, :].rearrange("p j (h w) -> p j h w", w=65)[:, :, :, 64:65],
                1.0)
            ocat_sb = ap.tile([64, HL, T], BF16, tag="ocat")
            fin_sb = ap.tile([128, MC, T], F32, tag="fin")

            # ---- projection emitters / granules ----
            def emit_xe(m, th):
                    p = ps_mm.tile([128, 1024], F32, tag="mm",
                                   name=f"xe_ps_{m}_{th}")
                    for t in range(2):
                        for k in range(KC):
                            nc.tensor.matmul(
                                p[:, t * 512:(t + 1) * 512],
                                w_inT_sb[:, k, m * 128:(m + 1) * 128],
                                xT_sb[:, k, th * 1024 + t * 512:
                                      th * 1024 + (t + 1) * 512],
                                start=(k == 0), stop=(k == KC - 1))
                    nc.scalar.activation(
                        out=xeT_sb[:, m, th * 1024:(th + 1) * 1024], in_=p[:, :],
                        func=AF.Identity, bias=b_in_sb[:, m:m + 1], scale=1.0)

            def gran_q(m, dve=False):
                gs = []
                for t in range(NT5):
                    p = ps_mm.tile([128, 512], F32, tag="mm",
                                   name=f"q_ps_{m}_{t}")
                    for k in range(KE):
                        gs.append(lambda p=p, t=t, k=k: nc.tensor.matmul(
                            p[:, :],
                            wqT_sb[:, k, m * 128:(m + 1) * 128],
                            xeT_sb[:, k, t * 512:(t + 1) * 512],
                            start=(k == 0), stop=(k == KE - 1)))
                    dst = QT_sb[:, m, t * 512:(t + 1) * 512]
                    if dve:
                        gs.append(lambda p=p, dst=dst: nc.vector.tensor_scalar_add(
                            dst, p[:, :], bq_sb[:, m:m + 1]))
                    else:
                        gs.append(lambda p=p, dst=dst: nc.scalar.activation(
                            out=dst, in_=p[:, :],
                            func=AF.Identity, bias=bq_sb[:, m:m + 1], scale=1.0))
                return gs

            def gran_k(m, dve=False):
                gs = []
                for sc in range(S // 512):
                    p = ps_mm.tile([128, 512], F32, tag="mm",
                                   name=f"k_ps_{m}_{sc}")
                    for k in range(KE):
                        gs.append(lambda p=p, sc=sc, k=k: nc.tensor.matmul(
                            p[:, :],
                            wkT_sb[:, k, m * 128:(m + 1) * 128],
                            ctxT_sb[:, k, sc * 512:(sc + 1) * 512],
                            start=(k == 0), stop=(k == KE - 1)))
                    dst = KT_sb[:, m, sc * 512:(sc + 1) * 512]
                    if dve:
                        gs.append(lambda p=p, dst=dst: nc.vector.tensor_scalar_add(
                            dst, p[:, :], bk_sb[:, m:m + 1]))
                    else:
                        gs.append(lambda p=p, dst=dst: nc.scalar.activation(
                            out=dst, in_=p[:, :],
                            func=AF.Identity, bias=bk_sb[:, m:m + 1], scale=1.0))
                return gs

            def emit_v(st):
                pv = ps_mm.tile([128, 512], F32, tag="mm")
                for k in range(KE):
                    nc.tensor.matmul(
                        pv[:, 0:EL],
                        ctxT_sb[:, k, st * 128:(st + 1) * 128],
                        wvT_sb[:, k, :],
                        start=(k == 0), stop=False)
                nc.tensor.matmul(pv[:, 0:EL], ones128[0:1, :], wvb_sb[0:1, :],
                                 start=False, stop=True)
                dst = V_sb[:, st, :].rearrange("p (h w) -> p h w", w=65)[:, :, 0:64]
                srcv = pv[:, 0:EL].rearrange("p (h w) -> p h w", w=64)
                nc.vector.tensor_copy(dst, srcv)

            def emit_unit(h, tc, bgq, pending):
                """One (head, T-chunk) attention unit. AV matmuls are carried
                one j-step behind (emitted after the NEXT step's QK) so the
                in-order PE stream never head-of-line blocks on an AV that
                waits for exp/mask; `pending` carries them across units."""
                et, bp = h // 2, 64 * (h % 2)
                t0 = tc * 1024
                oavs = [ps_av.tile([65, 512], F32, tag="av",
                                   name=f"oav_{h}_{tc}_{i}") for i in range(2)]
                for j in range(NJ):
                    pqk = ps_mm.tile([128, 1024], F32, tag="mm")
                    for t in range(2):
                        nc.tensor.matmul(
                            pqk[:, t * 512:(t + 1) * 512],
                            KT_sb[bp:bp + 64, et, j * 128:(j + 1) * 128],
                            QT_sb[bp:bp + 64, et, t0 + t * 512:t0 + (t + 1) * 512],
                            start=True, stop=True)
                    pe_t = pp.tile([128, 1024], BF16, tag="pexp")
                    nc.scalar.activation(out=pe_t[:, :], in_=pqk[:, :],
                                         func=AF.Exp, scale=SCALE)
                    pm_t = pp.tile([128, 1024], BF16, tag="pmask")
                    nc.vector.tensor_mul(pm_t[:, :], pe_t[:, :],
                                         zT_sb[:, j, t0:t0 + 1024])
                    while pending:
                        pending.popleft()()
                    def av(j=j, pm_t=pm_t):
                        for t in range(2):
                            nc.tensor.matmul(
                                oavs[t][:, :],
                                V_sb[:, j, h * 65:(h + 1) * 65],
                                pm_t[:, t * 512:(t + 1) * 512],
                                start=(j == 0), stop=(j == NJ - 1))
                        if bgq:
                            bgq.popleft()()
                    pending.append(av)

                def norm():
                    for t in range(2):
                        rec = nrm.tile([1, 512], F32, tag="rec")
                        nc.vector.reciprocal(rec[0:1, :], oavs[t][64:65, :])
                        rb = nrm.tile([64, 512], F32, tag="rb")
                        nc.gpsimd.partition_broadcast(rb[:, :], rec[0:1, :])
                        nc.vector.tensor_mul(
                            ocat_sb[0:64, h, t0 + t * 512:t0 + (t + 1) * 512],
                            oavs[t][0:64, :], rb[:, :])
                pending.append(norm)

            # ---- emission schedule ----
            for _rep in range(repeat):
                for th in range(2):
                    for m in range(KE):
                        emit_xe(m, th)
                for g in gran_q(0):
                    g()
                for g in gran_k(0):
                    g()
                for st in range(4):
                    emit_v(st)
                def gran_proj(tg):
                    """proj_out granules for T-half tg (needs all local heads'
                    ocat in that range): matmuls + eviction + output DMA."""
                    gs = []
                    for m in range(MC):
                        pf = ps_mm.tile([128, 1024], F32, tag="mm",
                                        name=f"pf_{m}_{tg}")
                        for t in range(2):
                            sl = slice((tg * 2 + t) * 512, (tg * 2 + t + 1) * 512)
                            for h in range(HL):
                                gs.append(lambda pf=pf, t=t, h=h, sl=sl, m=m:
                                          nc.tensor.matmul(
                                    pf[:, t * 512:(t + 1) * 512],
                                    w_outT_sb[0:64, h, m * 128:(m + 1) * 128],
                                    ocat_sb[0:64, h, sl],
                                    start=(h == 0), stop=False))
                            gs.append(lambda pf=pf, t=t, m=m: nc.tensor.matmul(
                                pf[:, t * 512:(t + 1) * 512],
                                wob_sb[0:1, m * 128:(m + 1) * 128],
                                onest[0:1, :], start=False, stop=True))
                        gs.append(lambda pf=pf, m=m, tg=tg: nc.vector.tensor_copy(
                            fin_sb[:, m, tg * 1024:(tg + 1) * 1024], pf[:, :]))
                    gs.append(lambda tg=tg: nc.sync.dma_start(
                        out=outT[:, :, tg * 1024:(tg + 1) * 1024],
                        in_=fin_sb[:, :, tg * 1024:(tg + 1) * 1024]))
                    return gs

                bgq = deque()
                for st in range(4, NJ):
                    bgq.append(lambda st=st: emit_v(st))
                bgq.extend(gran_q(1, dve=True))
                bgq.extend(gran_k(1, dve=True))
                pending = deque()
                for u, (h, tc) in enumerate([(h, tc) for h in range(HL)
                                             for tc in range(NTC)]):
                    if u == 7:
                        # last unit: background the first T-half's proj_out
                        bgq.extend(gran_proj(0))
                    emit_unit(h, tc, bgq, pending)
                while pending:
                    pending.popleft()()
                while bgq:
                    bgq.popleft()()
                for g in gran_proj(1):
                    g()

    nc.finalize()
    return nc


def get_nc(repeat=1):
    if repeat not in _NC_CACHE:
        _NC_CACHE[repeat] = _build_nc(repeat)
    return _NC_CACHE[repeat]


def _pack(a, p=128):
    """[k*p, f...] -> [p, k, f...] C-contiguous."""
    k = a.shape[0] // p
    return np.ascontiguousarray(
        a.reshape(k, p, *a.shape[1:]).transpose(1, 0, *range(2, a.ndim + 1)))


def build_in_maps(x, context, pad_mask, w_in, b_in, wq, bq, wk, bk, wv, bv,
                  w_out, b_out):
    bf = ml_dtypes.bfloat16
    f32 = np.float32

    w_inT_p = _pack(w_in.T.astype(bf))                    # [128, 2, 512]
    b_in_p = np.ascontiguousarray(b_in.reshape(KE, 128).T).astype(f32)
    wob_row = b_out[None, :].astype(bf)
    wob_zero = np.zeros_like(wob_row)
    z = (~pad_mask).astype(bf)                            # [B, T, S]

    # per-head-group weight slices
    wq_g, wk_g, wv_g, wvb_g, wo_g, bq_g, bk_g = [], [], [], [], [], [], []
    for hg in range(2):
        sl = slice(hg * EL, (hg + 1) * EL)
        wq_g.append(_pack(np.ascontiguousarray(wq.T[:, sl]).astype(bf)))
        wk_g.append(_pack(np.ascontiguousarray(wk.T[:, sl]).astype(bf)))
        wv_g.append(_pack(np.ascontiguousarray(wv.T[:, sl]).astype(bf)))
        wvb_g.append(bv[None, sl].astype(bf))
        wo_g.append(np.ascontiguousarray(
            w_out.T[sl, :].reshape(HL, 64, C).transpose(1, 0, 2)).astype(bf))
        bq_g.append(np.ascontiguousarray(bq[sl].reshape(ML, 128).T).astype(f32))
        bk_g.append(np.ascontiguousarray(bk[sl].reshape(ML, 128).T).astype(f32))

    in_maps = []
    for c in range(NCORES):
        b, hg = c // 2, c % 2
        xT_p = _pack(np.ascontiguousarray(x[b].T).astype(bf))      # [128,2,2048]
        ctxT_p = _pack(np.ascontiguousarray(context[b].T).astype(bf))
        zT_p = _pack(np.ascontiguousarray(z[b].T))                 # [128,16,2048]
        in_maps.append({
            "xT": xT_p, "ctxT": ctxT_p, "zT": zT_p,
            "w_inT": w_inT_p, "wqT": wq_g[hg], "wkT": wk_g[hg],
            "wvT": wv_g[hg], "wvb": wvb_g[hg], "w_outT": wo_g[hg],
            "wob": wob_row if hg == 0 else wob_zero,
            "b_in": b_in_p, "bq": bq_g[hg], "bk": bk_g[hg],
        })
    return in_maps


def assemble_output(results):
    out = np.empty((B, T, C), dtype=np.float32)
    for b in range(B):
        acc = results[2 * b]["outT"] + results[2 * b + 1]["outT"]  # [128,2,2048]
        ct = acc.transpose(1, 0, 2).reshape(C, T)
        out[b] = ct.T
    return out


def run(in_maps, repeat=1, **kw):
    return run_bass_kernel_spmd(get_nc(repeat), in_maps,
                                core_ids=list(range(NCORES)), **kw)


def kernel(**inputs):
    in_maps = build_in_maps(
        np.asarray(inputs["x"]), np.asarray(inputs["context"]),
        np.asarray(inputs["pad_mask"]), np.asarray(inputs["w_in"]),
        np.asarray(inputs["b_in"]), np.asarray(inputs["wq"]),
        np.asarray(inputs["bq"]), np.asarray(inputs["wk"]),
        np.asarray(inputs["bk"]), np.asarray(inputs["wv"]),
        np.asarray(inputs["bv"]), np.asarray(inputs["w_out"]),
        np.asarray(inputs["b_out"]))
    res = run(in_maps)
    return assemble_output(res.results)


# revision 38
# speedup vs baseline: 10.4664x; 10.4664x over previous
"""Trainium2 Bass kernel for Cross-MultiAttention.

Problem (hardcoded shapes):
  B=4, T=2048, S=2048, C=256, E=512, H=8 heads, D=64, SCALE=E**-0.5
  xe  = x @ w_in.T + b_in                  [B,T,C] -> [B,T,E]
  Q   = xe @ wq.T + bq;  K/V from context  [B,S,E]
  att = softmax(mask(QK^T * SCALE))        [B,H,T,S]
  out = (att @ V) @ w_out.T + b_out        -> [B,T,C]

Sharding: 8 cores. Core c handles batch c//2 and head group c%2 (4 heads),
for the full T=2048 query range (head/tensor parallel within each batch
pair). proj_in (xe) is replicated within a pair; Q/K/V/proj_out use the
core's head-slice of the weights; proj_out emits a PARTIAL [C, T] result
(row-sharded w_out) and the host sums the pair's partials.

Device algorithm per core (bf16 matmul inputs / fp32 PSUM accumulate):
  xeT  [E,T]    = w_inT.T @ xT   (+b_in via ACT bias)
  QT   [256,T]  = wqT.T @ xeT    (+bq)   (local heads' e-range)
  KT   [256,S]  = wkT.T @ ctxT   (+bk)
  V    [S,4*65] = ctxT.T @ wvT   (+bv via K=1 ones matmul; col h*65+64 is an
                  all-ones denominator column, memset once)
  per (head h<4, T-chunk tc<2, key-tile j<16):
     logitsT = KT_h_j.T @ QT_h_tc          (PE, [128, 1024])
     P  = exp(SCALE * logitsT)             (ACT; no max pass needed:
                                            |SCALE*logits| < ~1, masking is
                                            multiplicative)
     Pm = P * zT[j]                        (DVE; z = 1-mask bf16)
     oav[0:65] += V_j_h.T @ Pm             (PE; row 64 = softmax denominator)
  normalize: rec = 1/oav[64] (DVE); rb = bcast(rec) (GPSIMD); ocat = oav*rb
  finT [C,T] (partial) = w_outT.T @ ocat (+b_out on head-group-0 cores only)

Q/K projections for the second half of local heads run as background
granules, one PE matmul per attention j-step, to fill PE slack during the
ACT(exp)-bound attention window.
"""

from collections import deque

import numpy as np
import ml_dtypes

import concourse.bass as bass
import concourse.tile as tile
import concourse.mybir as mybir
from concourse.bacc import Bacc
from concourse.bass_utils import run_bass_kernel_spmd

BF16 = mybir.dt.bfloat16
F32 = mybir.dt.float32
AF = mybir.ActivationFunctionType

B, T, S, C, E, H = 4, 2048, 2048, 256, 512, 8
D = E // H
SCALE = float(E) ** -0.5
NCORES = 8
HL = H // 2                    # 4 local heads per core
EL = HL * D                    # 256 local head-dims
NJ = S // 128                  # 16 key tiles
KE = E // 128                  # 4 contraction tiles over E
KC = C // 128                  # 2 contraction tiles over C
ML = EL // 128                 # 2 local m-tiles (Q/K out)
MC = C // 128                  # 2 output c-tiles
NTC = T // 1024                # 2 attention T-chunks
NT5 = T // 512                 # 4 512-chunks
HW = HL * 65                   # 260 V columns incl. denominator cols

_NC_CACHE = {}


def _build_nc(repeat=1):
    nc = Bacc("TRN2", target_bir_lowering=False, debug=False)

    xT = nc.dram_tensor("xT", [128, KC, T], BF16, kind="ExternalInput")
    ctxT = nc.dram_tensor("ctxT", [128, KE, S], BF16, kind="ExternalInput")
    zT = nc.dram_tensor("zT", [128, NJ, T], BF16, kind="ExternalInput")
    w_inT = nc.dram_tensor("w_inT", [128, KC, E], BF16, kind="ExternalInput")
    wqT = nc.dram_tensor("wqT", [128, KE, EL], BF16, kind="ExternalInput")
    wkT = nc.dram_tensor("wkT", [128, KE, EL], BF16, kind="ExternalInput")
    wvT = nc.dram_tensor("wvT", [128, KE, EL], BF16, kind="ExternalInput")
    wvb = nc.dram_tensor("wvb", [1, EL], BF16, kind="ExternalInput")
    w_outT = nc.dram_tensor("w_outT", [64, HL, C], BF16, kind="ExternalInput")
    wob = nc.dram_tensor("wob", [1, C], BF16, kind="ExternalInput")
    b_in = nc.dram_tensor("b_in", [128, KE], F32, kind="ExternalInput")
    bq = nc.dram_tensor("bq", [128, ML], F32, kind="ExternalInput")
    bk = nc.dram_tensor("bk", [128, ML], F32, kind="ExternalInput")
    outT = nc.dram_tensor("outT", [128, MC, T], F32, kind="ExternalOutput")

    with tile.TileContext(nc) as tc:
        with tc.tile_pool(name="const", bufs=1) as cp, \
             tc.tile_pool(name="acts", bufs=1) as ap, \
             tc.tile_pool(name="pp", bufs=5) as pp, \
             tc.tile_pool(name="nrm", bufs=2) as nrm, \
             tc.tile_pool(name="mm", bufs=3, space="PSUM") as ps_mm, \
             tc.tile_pool(name="av", bufs=2, space="PSUM") as ps_av:

            # ---- persistent loads (early-needed first; zT big and last) ----
            w_inT_sb = cp.tile([128, KC, E], BF16, tag="w_inT")
            nc.sync.dma_start(out=w_inT_sb, in_=w_inT[:, :, :])
            xT_sb = cp.tile([128, KC, T], BF16, tag="xT")
            nc.sync.dma_start(out=xT_sb[:, :, 0:1024], in_=xT[:, :, 0:1024])
            nc.sync.dma_start(out=xT_sb[:, :, 1024:T], in_=xT[:, :, 1024:T])
            b_in_sb = cp.tile([128, KE], F32, tag="b_in")
            nc.sync.dma_start(out=b_in_sb, in_=b_in[:, :])
            bq_sb = cp.tile([128, ML], F32, tag="bq")
            nc.sync.dma_start(out=bq_sb, in_=bq[:, :])
            bk_sb = cp.tile([128, ML], F32, tag="bk")
            nc.sync.dma_start(out=bk_sb, in_=bk[:, :])
            wqT_sb = cp.tile([128, KE, EL], BF16, tag="wqT")
            nc.sync.dma_start(out=wqT_sb, in_=wqT[:, :, :])
            wkT_sb = cp.tile([128, KE, EL], BF16, tag="wkT")
            nc.sync.dma_start(out=wkT_sb, in_=wkT[:, :, :])
            ctxT_sb = cp.tile([128, KE, S], BF16, tag="ctxT")
            nc.sync.dma_start(out=ctxT_sb, in_=ctxT[:, :, :])
            wvT_sb = cp.tile([128, KE, EL], BF16, tag="wvT")
            nc.sync.dma_start(out=wvT_sb, in_=wvT[:, :, :])
            wvb_sb = cp.tile([1, EL], BF16, tag="wvb")
            nc.sync.dma_start(out=wvb_sb, in_=wvb[:, :])
            w_outT_sb = cp.tile([64, HL, C], BF16, tag="w_outT")
            nc.sync.dma_start(out=w_outT_sb, in_=w_outT[:, :, :])
            wob_sb = cp.tile([1, C], BF16, tag="wob")
            nc.sync.dma_start(out=wob_sb, in_=wob[:, :])
            # zT streamed per key-tile: TT(j) only needs chunk j, so the
            # attention window can start long before the full 8MB lands
            zT_sb = cp.tile([128, NJ, T], BF16, tag="zT")
            for j in range(NJ):
                nc.sync.dma_start(out=zT_sb[:, j, :], in_=zT[:, j, :])

            ones128 = cp.tile([1, 128], BF16, tag="ones128")
            nc.vector.memset(ones128, 1.0)
            onest = cp.tile([1, 512], BF16, tag="onest")
            nc.vector.memset(onest, 1.0)

            # ---- persistent activations ----
            xeT_sb = ap.tile([128, KE, T], BF16, tag="xeT")
            QT_sb = ap.tile([128, ML, T], BF16, tag="QT")
            KT_sb = ap.tile([128, ML, S], BF16, tag="KT")
            V_sb = ap.tile([128, NJ, HW], BF16, tag="V")
            nc.vector.memset(
                V_sb[:, :, :].rearrange("p j (h w) -> p j h w", w=65)[:, :, :, 64:65],
                1.0)
            ocat_sb = ap.tile([64, HL, T], BF16, tag="ocat")
            fin_sb = ap.tile([128, MC, T], F32, tag="fin")

            # ---- projection emitters / granules ----
            def emit_xe(m, th):
                    p = ps_mm.tile([128, 1024], F32, tag="mm",
                                   name=f"xe_ps_{m}_{th}")
                    for t in range(2):
                        for k in range(KC):
                            nc.tensor.matmul(
                                p[:, t * 512:(t + 1) * 512],
                                w_inT_sb[:, k, m * 128:(m + 1) * 128],
                                xT_sb[:, k, th * 1024 + t * 512:
                                      th * 1024 + (t + 1) * 512],
                                start=(k == 0), stop=(k == KC - 1))
                    nc.scalar.activation(
                        out=xeT_sb[:, m, th * 1024:(th + 1) * 1024], in_=p[:, :],
                        func=AF.Identity, bias=b_in_sb[:, m:m + 1], scale=1.0)

            def gran_q(m, dve=False):
                gs = []
                for t in range(NT5):
                    p = ps_mm.tile([128, 512], F32, tag="mm",
                                   name=f"q_ps_{m}_{t}")
                    for k in range(KE):
                        gs.append(lambda p=p, t=t, k=k: nc.tensor.matmul(
                            p[:, :],
                            wqT_sb[:, k, m * 128:(m + 1) * 128],
                            xeT_sb[:, k, t * 512:(t + 1) * 512],
                            start=(k == 0), stop=(k == KE - 1)))
                    dst = QT_sb[:, m, t * 512:(t + 1) * 512]
                    if dve:
                        gs.append(lambda p=p, dst=dst: nc.vector.tensor_scalar_add(
                            dst, p[:, :], bq_sb[:, m:m + 1]))
                    else:
                        gs.append(lambda p=p, dst=dst: nc.scalar.activation(
                            out=dst, in_=p[:, :],
                            func=AF.Identity, bias=bq_sb[:, m:m + 1], scale=1.0))
                return gs

            def gran_k(m, dve=False):
                gs = []
                for sc in range(S // 512):
                    p = ps_mm.tile([128, 512], F32, tag="mm",
                                   name=f"k_ps_{m}_{sc}")
                    for k in range(KE):
                        gs.append(lambda p=p, sc=sc, k=k: nc.tensor.matmul(
                            p[:, :],
                            wkT_sb[:, k, m * 128:(m + 1) * 128],
                            ctxT_sb[:, k, sc * 512:(sc + 1) * 512],
                            start=(k == 0), stop=(k == KE - 1)))
                    dst = KT_sb[:, m, sc * 512:(sc + 1) * 512]
                    if dve:
                        gs.append(lambda p=p, dst=dst: nc.vector.tensor_scalar_add(
                            dst, p[:, :], bk_sb[:, m:m + 1]))
                    else:
                        gs.append(lambda p=p, dst=dst: nc.scalar.activation(
                            out=dst, in_=p[:, :],
                            func=AF.Identity, bias=bk_sb[:, m:m + 1], scale=1.0))
                return gs

            def emit_v(st):
                pv = ps_mm.tile([128, 512], F32, tag="mm")
                for k in range(KE):
                    nc.tensor.matmul(
                        pv[:, 0:EL],
                        ctxT_sb[:, k, st * 128:(st + 1) * 128],
                        wvT_sb[:, k, :],
                        start=(k == 0), stop=False)
                nc.tensor.matmul(pv[:, 0:EL], ones128[0:1, :], wvb_sb[0:1, :],
                                 start=False, stop=True)
                dst = V_sb[:, st, :].rearrange("p (h w) -> p h w", w=65)[:, :, 0:64]
                srcv = pv[:, 0:EL].rearrange("p (h w) -> p h w", w=64)
                nc.vector.tensor_copy(dst, srcv)

            def emit_unit(h, tc, bgq, pending):
                """One (head, T-chunk) attention unit. AV matmuls are carried
                one j-step behind (emitted after the NEXT step's QK) so the
                in-order PE stream never head-of-line blocks on an AV that
                waits for exp/mask; `pending` carries them across units."""
                et, bp = h // 2, 64 * (h % 2)
                t0 = tc * 1024
                oavs = [ps_av.tile([65, 512], F32, tag="av",
                                   name=f"oav_{h}_{tc}_{i}") for i in range(2)]
                for j in range(NJ):
                    pqk = ps_mm.tile([128, 1024], F32, tag="mm")
                    for t in range(2):
                        nc.tensor.matmul(
                            pqk[:, t * 512:(t + 1) * 512],
                            KT_sb[bp:bp + 64, et, j * 128:(j + 1) * 128],
                            QT_sb[bp:bp + 64, et, t0 + t * 512:t0 + (t + 1) * 512],
                            start=True, stop=True)
                    pe_t = pp.tile([128, 1024], BF16, tag="pexp")
                    nc.scalar.activation(out=pe_t[:, :], in_=pqk[:, :],
                                         func=AF.Exp, scale=SCALE)
                    pm_t = pp.tile([128, 1024], BF16, tag="pmask")
                    nc.vector.tensor_mul(pm_t[:, :], pe_t[:, :],
                                         zT_sb[:, j, t0:t0 + 1024])
                    while pending:
                        pending.popleft()()
                    def av(j=j, pm_t=pm_t):
                        for t in range(2):
                            nc.tensor.matmul(
                                oavs[t][:, :],
                                V_sb[:, j, h * 65:(h + 1) * 65],
                                pm_t[:, t * 512:(t + 1) * 512],
                                start=(j == 0), stop=(j == NJ - 1))
                        if bgq:
                            bgq.popleft()()
                    pending.append(av)

                def norm():
                    for t in range(2):
                        rec = nrm.tile([1, 512], F32, tag="rec")
                        nc.vector.reciprocal(rec[0:1, :], oavs[t][64:65, :])
                        rb = nrm.tile([64, 512], F32, tag="rb")
                        nc.gpsimd.partition_broadcast(rb[:, :], rec[0:1, :])
                        nc.vector.tensor_mul(
                            ocat_sb[0:64, h, t0 + t * 512:t0 + (t + 1) * 512],
                            oavs[t][0:64, :], rb[:, :])
                pending.append(norm)

            # ---- emission schedule ----
            for _rep in range(repeat):
                for th in range(2):
                    for m in range(KE):
                        emit_xe(m, th)
                for g in gran_q(0):
                    g()
                for g in gran_k(0):
                    g()
                for st in range(4):
                    emit_v(st)
                def gran_proj(tg):
                    """proj_out granules for T-half tg (needs all local heads'
                    ocat in that range): matmuls + eviction + output DMA."""
                    gs = []
                    for m in range(MC):
                        pf = ps_mm.tile([128, 1024], F32, tag="mm",
                                        name=f"pf_{m}_{tg}")
                        for t in range(2):
                            sl = slice((tg * 2 + t) * 512, (tg * 2 + t + 1) * 512)
                            for h in range(HL):
                                gs.append(lambda pf=pf, t=t, h=h, sl=sl, m=m:
                                          nc.tensor.matmul(
                                    pf[:, t * 512:(t + 1) * 512],
                                    w_outT_sb[0:64, h, m * 128:(m + 1) * 128],
                                    ocat_sb[0:64, h, sl],
                                    start=(h == 0), stop=False))
                            gs.append(lambda pf=pf, t=t, m=m: nc.tensor.matmul(
                                pf[:, t * 512:(t + 1) * 512],
                                wob_sb[0:1, m * 128:(m + 1) * 128],
                                onest[0:1, :], start=False, stop=True))
                        gs.append(lambda pf=pf, m=m, tg=tg: nc.vector.tensor_copy(
                            fin_sb[:, m, tg * 1024:(tg + 1) * 1024], pf[:, :]))
                    gs.append(lambda tg=tg: nc.sync.dma_start(
                        out=outT[:, :, tg * 1024:(tg + 1) * 1024],
                        in_=fin_sb[:, :, tg * 1024:(tg + 1) * 1024]))
                    return gs

                bgq = deque()
                for st in range(4, NJ):
                    bgq.append(lambda st=st: emit_v(st))
                bgq.extend(gran_q(1, dve=True))
                bgq.extend(gran_k(1, dve=True))
                pending = deque()
                for u, (h, tc) in enumerate([(h, tc) for h in range(HL)
                                             for tc in range(NTC)]):
                    if u == 7:
                        # last unit: background the first T-half's proj_out
                        bgq.extend(gran_proj(0))
                    emit_unit(h, tc, bgq, pending)
                while pending:
                    pending.popleft()()
                while bgq:
                    bgq.popleft()()
                for g in gran_proj(1):
                    g()

    nc.finalize()
    return nc


def get_nc(repeat=1):
    if repeat not in _NC_CACHE:
        _NC_CACHE[repeat] = _build_nc(repeat)
    return _NC_CACHE[repeat]


def _pack(a, p=128):
    """[k*p, f...] -> [p, k, f...] C-contiguous."""
    k = a.shape[0] // p
    return np.ascontiguousarray(
        a.reshape(k, p, *a.shape[1:]).transpose(1, 0, *range(2, a.ndim + 1)))


def build_in_maps(x, context, pad_mask, w_in, b_in, wq, bq, wk, bk, wv, bv,
                  w_out, b_out):
    bf = ml_dtypes.bfloat16
    f32 = np.float32

    w_inT_p = _pack(w_in.T.astype(bf))                    # [128, 2, 512]
    b_in_p = np.ascontiguousarray(b_in.reshape(KE, 128).T).astype(f32)
    wob_row = b_out[None, :].astype(bf)
    wob_zero = np.zeros_like(wob_row)
    z = (~pad_mask).astype(bf)                            # [B, T, S]

    # per-head-group weight slices
    wq_g, wk_g, wv_g, wvb_g, wo_g, bq_g, bk_g = [], [], [], [], [], [], []
    for hg in range(2):
        sl = slice(hg * EL, (hg + 1) * EL)
        wq_g.append(_pack(np.ascontiguousarray(wq.T[:, sl]).astype(bf)))
        wk_g.append(_pack(np.ascontiguousarray(wk.T[:, sl]).astype(bf)))
        wv_g.append(_pack(np.ascontiguousarray(wv.T[:, sl]).astype(bf)))
        wvb_g.append(bv[None, sl].astype(bf))
        wo_g.append(np.ascontiguousarray(
            w_out.T[sl, :].reshape(HL, 64, C).transpose(1, 0, 2)).astype(bf))
        bq_g.append(np.ascontiguousarray(bq[sl].reshape(ML, 128).T).astype(f32))
        bk_g.append(np.ascontiguousarray(bk[sl].reshape(ML, 128).T).astype(f32))

    in_maps = []
    for c in range(NCORES):
        b, hg = c // 2, c % 2
        xT_p = _pack(np.ascontiguousarray(x[b].T).astype(bf))      # [128,2,2048]
        ctxT_p = _pack(np.ascontiguousarray(context[b].T).astype(bf))
        zT_p = _pack(np.ascontiguousarray(z[b].T))                 # [128,16,2048]
        in_maps.append({
            "xT": xT_p, "ctxT": ctxT_p, "zT": zT_p,
            "w_inT": w_inT_p, "wqT": wq_g[hg], "wkT": wk_g[hg],
            "wvT": wv_g[hg], "wvb": wvb_g[hg], "w_outT": wo_g[hg],
            "wob": wob_row if hg == 0 else wob_zero,
            "b_in": b_in_p, "bq": bq_g[hg], "bk": bk_g[hg],
        })
    return in_maps


def assemble_output(results):
    out = np.empty((B, T, C), dtype=np.float32)
    for b in range(B):
        acc = results[2 * b]["outT"] + results[2 * b + 1]["outT"]  # [128,2,2048]
        ct = acc.transpose(1, 0, 2).reshape(C, T)
        out[b] = ct.T
    return out


def run(in_maps, repeat=1, **kw):
    return run_bass_kernel_spmd(get_nc(repeat), in_maps,
                                core_ids=list(range(NCORES)), **kw)


def kernel(**inputs):
    in_maps = build_in_maps(
        np.asarray(inputs["x"]), np.asarray(inputs["context"]),
        np.asarray(inputs["pad_mask"]), np.asarray(inputs["w_in"]),
        np.asarray(inputs["b_in"]), np.asarray(inputs["wq"]),
        np.asarray(inputs["bq"]), np.asarray(inputs["wk"]),
        np.asarray(inputs["bk"]), np.asarray(inputs["wv"]),
        np.asarray(inputs["bv"]), np.asarray(inputs["w_out"]),
        np.asarray(inputs["b_out"]))
    res = run(in_maps)
    return assemble_output(res.results)
